# revision 23
# baseline (speedup 1.0000x reference)
"""CoPE-with-FIRE fused kernel for 8 Trainium2 NeuronCores.

Math (per head h, per query row q, over key axis j):
    g    = sigmoid(logits)                       [S]
    pos  = reverse-cumsum(g)                     [S]   (suffix sums)
    num  = ln(1 + c*pos)
    den  = ln(1 + c*min(pos[0], thr)) + EPS      (pos[0] = row total)
    d    = num / den                             in (0, ~1.1]
    out  = b_out[h] + sum_w W_out[h,w]*relu(w1[w]*d + b_in[w])

The MLP is a 32-knot piecewise-linear function of d.  Hidden units whose knot
t_w = -b_in/w1 lies outside the reachable range (0, dmax] are always-on or
always-off, so the host folds them into a per-head affine A + B*d.  The ~18
remaining "active" units are evaluated as sign*relu(a*d + c) with a, c, sign
per (head, unit), streamed as [P,1] scalars (one SPMD program for all cores).

Sharding: rows (h, q) flattened to [9216, 768], 1152 rows per core.  Each
128-row tile lies in one head, and each core's 9 tiles always split 6+3 over
exactly two heads; the host permutes each core's tiles so the layout is
uniformly [6-tile group A | 3-tile group B], letting phase-B ops run per
group with per-group [P,1] MLP params.

mode="exact":  per active unit: one ACT Relu pass (scale/bias APs) + one DVE
               scalar_tensor_tensor accumulate pass over the full data.
mode="interp": evaluate f exactly only at static sample columns, then
               secant-interpolate in num-space inside each inter-sample block
               (exact wherever no knot is crossed inside the block).
"""

import numpy as np

EPS = 1e-06
B, H, S, W = 1, 12, 768, 32
NCORES = 8
P = 128
ROWS_PER_CORE = H * S // NCORES          # 1152
NT = ROWS_PER_CORE // P                  # 9 tiles/core
TILES_PER_HEAD = S // P                  # 6
GROUPS = (6, 3)                          # tiles per group after permutation

_CACHE = {}
_last_in_maps = None


# --------------------------------------------------------------------------- #
# host-side parameter folding
# --------------------------------------------------------------------------- #
def _fold_mlp(W_in, b_in, W_out, b_out, c, thr):
    """Returns (act_idx[K], A[H], Bc[H], a[H,K], cc[H,K], sg[H,K]) float64."""
    w1 = W_in[:, 0].astype(np.float64)
    b = b_in.astype(np.float64)
    Wo = W_out.astype(np.float64)
    dmax = max(1.0, np.log1p(c * S) / np.log1p(c * min(S, thr))) + 1e-6
    A = b_out.astype(np.float64).copy()
    Bc = np.zeros(H, np.float64)
    act = []
    for w in range(W):
        if w1[w] == 0.0:
            A += Wo[:, w] * max(b[w], 0.0)
            continue
        t = -b[w] / w1[w]
        always_on = (w1[w] > 0 and t <= 0.0) or (w1[w] < 0 and t >= dmax)
        never_on = (w1[w] > 0 and t >= dmax) or (w1[w] < 0 and t <= 0.0)
        if always_on:
            A += Wo[:, w] * b[w]
            Bc += Wo[:, w] * w1[w]
        elif not never_on:
            act.append(w)
    act = np.array(act, int)
    # term_w = sign(wout)*relu(|wout|*w1*d + |wout|*b)
    aw = np.abs(Wo[:, act]) * w1[act]          # [H, K]
    cw = np.abs(Wo[:, act]) * b[act]           # [H, K]
    sw = np.sign(Wo[:, act])                   # [H, K]
    knots = -b[act] / w1[act]
    order = np.argsort(knots)
    return knots[order], A, Bc, aw[:, order], cw[:, order], sw[:, order], act[order]


def _mlp_ref(d, h, W_in, b_in, W_out, b_out):
    z = d[..., None] * W_in[:, 0].astype(np.float64) + b_in.astype(np.float64)
    return np.maximum(z, 0.0) @ W_out[h].astype(np.float64) + float(b_out[h])


def _fold_eval(d, h, A, Bc, aw, cw, sw):
    f = A[h] + Bc[h] * d
    for k in range(aw.shape[1]):
        f = f + sw[h, k] * np.maximum(aw[h, k] * d + cw[h, k], 0.0)
    return f


# --------------------------------------------------------------------------- #
# wait legalization: this walrus codegen accepts at most ONE sync-wait per
# instruction.  Hoist excess waits onto injected same-engine NoOps (the engine
# blocks until they clear before issuing the original instruction).
# --------------------------------------------------------------------------- #
def _legalize_waits(nc):
    from concourse import mybir

    ctr = 0
    for f in nc.m.functions:
        for blk in f.blocks:
            insts = blk.instructions
            out = []
            changed = False
            for inst in insts:
                si = inst.sync_info
                waits = list(si.on_wait) if (si is not None and si.on_wait) else []
                if len(waits) <= 1:
                    out.append(inst)
                    continue
                for wcond in waits[:-1]:
                    ctr += 1
                    nop = mybir.InstNoOp(name=f"I-waitnop-{ctr}")
                    nop.engine = inst.engine
                    nop.sync_info = mybir.SyncInfo(on_wait=[wcond], on_update=[])
                    out.append(nop)
                si.on_wait = waits[-1:]
                out.append(inst)
                changed = True
            if changed:
                blk.instructions = out
    return nc


# --------------------------------------------------------------------------- #
# bass program
# --------------------------------------------------------------------------- #
def _build_program(K, mode, samples=None, legalize=True):
    import concourse.bass as bass
    import concourse.tile as tile
    from concourse import mybir
    from concourse.bass import _add_dep_helper

    f32 = mybir.dt.float32
    AF = mybir.ActivationFunctionType
    OP = mybir.AluOpType

    c = 0.1
    thr = 512.0
    NPG = 2 + 3 * K  # per-group params: A, B, a[K], c[K], s[K]

    nc = bass.Bass()
    x = nc.declare_dram_parameter("x", [ROWS_PER_CORE, S], f32, isOutput=False)
    pp = nc.declare_dram_parameter("pp", [P, 2 * NPG], f32, isOutput=False)
    y = nc.declare_dram_parameter("y", [ROWS_PER_CORE, S], f32, isOutput=True)

    with tile.TileContext(nc) as tc:
        with (
            tc.tile_pool(name="const", bufs=1) as const_pool,
            tc.tile_pool(name="io", bufs=2) as io_pool,
            tc.tile_pool(name="gt", bufs=2) as g_pool,
            tc.tile_pool(name="pos", bufs=2) as pos_pool,
            tc.tile_pool(name="big", bufs=1) as big_pool,
            tc.tile_pool(name="rp", bufs=2) as r_pool,
            tc.tile_pool(name="acc", bufs=2) as acc_pool,
            tc.tile_pool(name="sm", bufs=2) as sm_pool,
        ):
            params = const_pool.tile([P, 2 * NPG], f32)
            nc.gpsimd.dma_start(params[:], pp[:])
            negones = const_pool.tile([P, S], f32)
            nc.vector.memset(negones[:], -1.0)
            totals = const_pool.tile([P, NT], f32)
            recips = const_pool.tile([P, NT], f32)
            dsc = const_pool.tile([P, 2 * NT], f32)

            def prm(gi, k):  # [P,1] scalar AP for param k of group gi
                return params[:, gi * NPG + k : gi * NPG + k + 1]

            # ---- phase A: sigmoid + suffix-sum (sigmoid table set) ----
            pos_g = []
            sig_insts = []
            t0 = 0
            for gi, gn in enumerate(GROUPS):
                lt = io_pool.tile([P, gn * S], f32, tag="in")
                nc.gpsimd.dma_start(
                    lt[:].rearrange("p (t s) -> p t s", s=S),
                    x[t0 * P : (t0 + gn) * P, :].rearrange(
                        "(t p) s -> p t s", p=P
                    ),
                )
                pos = pos_pool.tile([P, gn * S], f32, tag="pos")
                for ti in range(gn):
                    t = t0 + ti
                    g = g_pool.tile([P, S], f32, tag="g")
                    sig = nc.scalar.activation(
                        g[:], lt[:, ti * S : (ti + 1) * S], AF.Sigmoid,
                        accum_out=totals[:, t : t + 1],
                    )
                    sig_insts.append(sig)
                    ps = pos[:, ti * S : (ti + 1) * S]
                    nc.vector.tensor_copy(ps[:, 0:1], totals[:, t : t + 1])
                    # pos[j] = total - sum_{k<j} g[k]:
                    #   state' = (g - state)*(-1),  state0 = total
                    nc.vector.tensor_tensor_scan(
                        ps[:, 1:S], g[:, 0 : S - 1], negones[:, 0 : S - 1],
                        totals[:, t : t + 1], OP.subtract, OP.mult,
                    )
                pos_g.append(pos)
                t0 += gn

            # ---- phase B: ln + MLP (natural_log table set) ----
            def dep(inst):
                _add_dep_helper(inst.ins, sig_insts[-1].ins, reason="ACT set order")
                return inst

            # per-tile 1/den
            for t in range(NT):
                nc.vector.tensor_scalar_min(
                    dsc[:, t : t + 1], totals[:, t : t + 1], thr
                )
                dep(nc.scalar.activation(
                    dsc[:, NT + t : NT + t + 1], dsc[:, t : t + 1], AF.Ln,
                    bias=1.0, scale=c,
                ))
                nc.vector.tensor_scalar_add(
                    dsc[:, t : t + 1], dsc[:, NT + t : NT + t + 1], EPS
                )
                nc.vector.reciprocal(
                    recips[:, t : t + 1], dsc[:, t : t + 1]
                )

            t0 = 0
            for gi, gn in enumerate(GROUPS):
                FD = gn * S
                pos = pos_g[gi]
                num = big_pool.tile([P, FD], f32, tag="num")
                dep(nc.scalar.activation(num[:], pos[:], AF.Ln, bias=1.0, scale=c))

                if mode == "exact":
                    dist = big_pool.tile([P, FD], f32, tag="dist")
                    for ti in range(gn):
                        t = t0 + ti
                        nc.vector.tensor_scalar_mul(
                            dist[:, ti * S : (ti + 1) * S],
                            num[:, ti * S : (ti + 1) * S],
                            recips[:, t : t + 1],
                        )
                    acc = acc_pool.tile([P, FD], f32, tag="acc")
                    nc.vector.tensor_scalar(
                        acc[:], dist[:], prm(gi, 1), prm(gi, 0), OP.mult, OP.add
                    )
                    for k in range(K):
                        r = r_pool.tile([P, FD], f32, tag="r")
                        dep(nc.scalar.activation(
                            r[:], dist[:], AF.Relu,
                            bias=prm(gi, 2 + K + k), scale=prm(gi, 2 + k),
                        ))
                        nacc = acc_pool.tile([P, FD], f32, tag="acc")
                        nc.vector.scalar_tensor_tensor(
                            nacc[:], r[:], prm(gi, 2 + 2 * K + k), acc[:],
                            OP.mult, OP.add,
                        )
                        acc = nacc
                    out_g = acc
                else:
                    out_g = _emit_interp_group(
                        nc, tc, mybir, dep, gi, gn, t0, num, recips, prm, K,
                        samples, big_pool, acc_pool, sm_pool, r_pool,
                    )

                nc.gpsimd.dma_start(
                    y[t0 * P : (t0 + gn) * P, :].rearrange(
                        "(t p) s -> p t s", p=P
                    ),
                    out_g[:].rearrange("p (t s) -> p t s", s=S),
                )
                t0 += gn
    return _legalize_waits(nc) if legalize else nc


def _emit_interp_group(
    nc, tc, mybir, dep, gi, gn, t0, num, recips, prm, K, samples,
    big_pool, acc_pool, sm_pool, r_pool,
):
    """Secant interpolation in num-space between static sample columns."""
    OP = mybir.AluOpType
    AF = mybir.ActivationFunctionType
    f32 = mybir.dt.float32
    ns = len(samples)
    nb = ns - 1
    FD = gn * S

    # widths of interp blocks; last block extended to cover the final column
    widths = [samples[k + 1] - samples[k] for k in range(nb)]
    widths[-1] = S - samples[nb - 1]

    # ---- gather sample columns of num into [P, gn*ns] (stride-run copies) --
    smp = sm_pool.tile([P, 4 * gn * ns], f32, tag="smp")
    num_s = smp[:, 0 : gn * ns]
    num3 = num[:].rearrange("p (t s) -> p t s", s=S)
    ns3 = num_s.rearrange("p (t s) -> p t s", s=ns)
    i = 0
    while i < ns:
        j = i + 1
        st = 1 if j >= ns else samples[j] - samples[i]
        while j < ns and samples[j] - samples[j - 1] == st:
            j += 1
        cnt = j - i
        if st > 1:
            src = num3[:, :, samples[i] : samples[i] + (cnt - 1) * st + 1 : st]
        else:
            src = num3[:, :, samples[i] : samples[i] + cnt]
        nc.vector.tensor_copy(ns3[:, :, i : i + cnt], src)
        i = j

    # ---- d at samples (per-tile recip), f at samples (exact eval) ----------
    d_s = smp[:, gn * ns : 2 * gn * ns]
    d3 = d_s.rearrange("p (t s) -> p t s", s=ns)
    for ti in range(gn):
        nc.vector.tensor_scalar_mul(
            d3[:, ti, :], ns3[:, ti, :], recips[:, t0 + ti : t0 + ti + 1]
        )
    fA = smp[:, 2 * gn * ns : 3 * gn * ns]
    fB = smp[:, 3 * gn * ns : 4 * gn * ns]
    nc.vector.tensor_scalar(fA, d_s, prm(gi, 1), prm(gi, 0), OP.mult, OP.add)
    f_cur = fA
    for k in range(K):
        r = r_pool.tile([P, gn * ns], f32, tag="rs")
        dep(nc.scalar.activation(
            r[:], d_s, AF.Relu, bias=prm(gi, 2 + K + k), scale=prm(gi, 2 + k)
        ))
        f_new = fB if f_cur is fA else fA
        nc.vector.scalar_tensor_tensor(
            f_new, r[:], prm(gi, 2 + 2 * K + k), f_cur, OP.mult, OP.add
        )
        f_cur = f_new

    # ---- secant coefficients per block --------------------------------------
    # Q = (f1-f0)/(n1-n0), Pc = f0 - Q*n0
    bl = sm_pool.tile([P, 4 * gn * nb], f32, tag="bl")
    f3 = f_cur.rearrange("p (t s) -> p t s", s=ns)
    dn = bl[:, 0 : gn * nb].rearrange("p (t s) -> p t s", s=nb)
    nc.vector.tensor_tensor(dn, ns3[:, :, 1:ns], ns3[:, :, 0:nb], OP.subtract)
    nc.vector.tensor_scalar_add(dn, dn, -1e-12)  # num strictly decreasing
    rdn = bl[:, gn * nb : 2 * gn * nb]
    nc.vector.reciprocal(rdn, bl[:, 0 : gn * nb])
    df = bl[:, 2 * gn * nb : 3 * gn * nb].rearrange("p (t s) -> p t s", s=nb)
    nc.vector.tensor_tensor(df, f3[:, :, 1:ns], f3[:, :, 0:nb], OP.subtract)
    Q = bl[:, 0 : gn * nb]  # overwrites dn
    nc.vector.tensor_tensor(
        Q, bl[:, 2 * gn * nb : 3 * gn * nb], rdn, OP.mult
    )
    QN = bl[:, 3 * gn * nb : 4 * gn * nb]
    QN3 = QN.rearrange("p (t s) -> p t s", s=nb)
    nc.vector.tensor_tensor(QN3, Q3_ := Q.rearrange("p (t s) -> p t s", s=nb),
                            ns3[:, :, 0:nb], OP.mult)
    Pc = bl[:, gn * nb : 2 * gn * nb]  # overwrites rdn
    Pc3 = Pc.rearrange("p (t s) -> p t s", s=nb)
    nc.vector.tensor_tensor(Pc3, f3[:, :, 0:nb], QN3, OP.subtract)

    # ---- out = Pc[blk] + Q[blk]*num, per (tile, equal-width run) -----------
    out_g = acc_pool.tile([P, FD], f32, tag="acc")
    o3 = out_g[:].rearrange("p (t s) -> p t s", s=S)
    Q3 = Q.rearrange("p (t s) -> p t s", s=nb)
    P3 = Pc.rearrange("p (t s) -> p t s", s=nb)
    for ti in range(gn):
        i = 0
        while i < nb:
            wdt = widths[i]
            j = i
            while j < nb and widths[j] == wdt:
                j += 1
            cnt = j - i
            j0 = samples[i]
            j1 = j0 + cnt * wdt
            ov = o3[:, ti, j0:j1].rearrange("p (n l) -> p n l", l=wdt)
            nv = num3[:, ti, j0:j1].rearrange("p (n l) -> p n l", l=wdt)
            qb = Q3[:, ti, i:j].unsqueeze(2).broadcast_to([P, cnt, wdt])
            pb = P3[:, ti, i:j].unsqueeze(2).broadcast_to([P, cnt, wdt])
            nc.vector.tensor_tensor(ov, nv, qb, OP.mult)
            nc.vector.tensor_tensor(ov, ov, pb, OP.add)
            i = j
    return out_g


# --------------------------------------------------------------------------- #
# sample schedule for mode="interp"
# --------------------------------------------------------------------------- #
def _make_samples(knots, cmax, c=0.1, tol=4e-4, den_nom=None, base_stride=64):
    """Knot-aware static block-edge schedule (see module docstring)."""
    if den_nom is None:
        den_nom = np.log1p(c * 0.5 * S)
    lim = np.full(S + 1, base_stride, np.int64)
    for k in range(len(knots)):
        ck = float(cmax[k]) + 1e-12
        pos_k = (np.exp(knots[k] * den_nom) - 1.0) / c
        m_k = 2.0 * pos_k
        m_lo = max(1, int(0.55 * m_k) - 8)
        m_hi = min(S, int(1.75 * m_k) + 10)
        for m in range(m_lo, m_hi + 1):
            pos_lo = 0.35 * m
            L = int(2.0 * tol * (1.0 + c * pos_lo) * den_nom / (c * ck))
            L = max(1, min(base_stride, L))
            L = 1 << (L.bit_length() - 1)
            lim[m] = min(lim[m], L)
    edges = [S - 1]
    j = S - 1
    while j > 0:
        m = S - j
        st = int(lim[min(m, S)])
        st = min(st, j)
        while st > 1 and int(lim[min(S - (j - st), S)]) < st:
            st //= 2
        j -= st
        edges.append(j)
    return sorted(edges)


# --------------------------------------------------------------------------- #
# entry point
# --------------------------------------------------------------------------- #
def _core_tile_order(cidx):
    """Global tile ids for core cidx, permuted to [6 of head A | 3 of head B]."""
    tiles = list(range(cidx * NT, (cidx + 1) * NT))
    byhead = {}
    for g in tiles:
        byhead.setdefault(g // TILES_PER_HEAD, []).append(g)
    (hA, tA), (hB, tB) = sorted(byhead.items(), key=lambda kv: -len(kv[1]))
    assert len(tA) == 6 and len(tB) == 3
    return tA + tB, hA, hB


def kernel(attn_logits, W_in, b_in, W_out, b_out, c, L_multiplier, init_L,
           mode="interp"):
    from concourse.bass_utils import run_bass_kernel_spmd

    attn_logits = np.asarray(attn_logits)
    W_in = np.asarray(W_in); b_in = np.asarray(b_in)
    W_out = np.asarray(W_out); b_out = np.asarray(b_out)
    cf = float(np.asarray(c))
    thr = abs(float(np.asarray(L_multiplier)) * float(np.asarray(init_L)))
    assert attn_logits.shape == (B, H, S, S)
    assert abs(cf - 0.1) < 1e-6 and abs(thr - 512.0) < 1e-3, "immediates baked"

    knots, A, Bc, aw, cw, sw, act = _fold_mlp(W_in, b_in, W_out, b_out, cf, thr)
    K = len(knots)
    d_chk = np.random.default_rng(0).uniform(0, 1.1, 256)
    for h in (0, H - 1):
        assert np.allclose(
            _fold_eval(d_chk, h, A, Bc, aw, cw, sw),
            _mlp_ref(d_chk, h, W_in, b_in, W_out, b_out), atol=1e-10,
        ), "MLP fold mismatch"

    if mode == "interp":
        cmax = (np.abs(W_out[:, act].astype(np.float64))
                * np.abs(W_in[act, 0].astype(np.float64))).max(axis=0) / 2.0
        samples = _make_samples(knots, cmax)
    else:
        samples = None
    key = (mode, K, tuple(samples) if samples else None)
    if key not in _CACHE:
        _CACHE[key] = _build_program(K, mode, samples)
    nc = _CACHE[key]

    xs = attn_logits.reshape(H * S, S).astype(np.float32)
    NPG = 2 + 3 * K
    in_maps = []
    orders = []
    for cidx in range(NCORES):
        order, hA, hB = _core_tile_order(cidx)
        orders.append(order)
        xr = np.concatenate(
            [xs[g * P : (g + 1) * P] for g in order], axis=0
        )
        prm_np = np.zeros((2, NPG), np.float32)
        for gi, h in enumerate((hA, hB)):
            prm_np[gi, 0] = A[h]
            prm_np[gi, 1] = Bc[h]
            prm_np[gi, 2 : 2 + K] = aw[h]
            prm_np[gi, 2 + K : 2 + 2 * K] = cw[h]
            prm_np[gi, 2 + 2 * K : 2 + 3 * K] = sw[h]
        in_maps.append({
            "x": np.ascontiguousarray(xr),
            "pp": np.ascontiguousarray(
                np.broadcast_to(prm_np.reshape(1, -1), (P, 2 * NPG))
            ),
        })

    global _last_in_maps
    _last_in_maps = in_maps
    res = run_bass_kernel_spmd(nc, in_maps, list(range(NCORES)))
    out = np.empty((H * S, S), np.float32)
    for cidx in range(NCORES):
        yc = res.results[cidx]["y"]
        for ti, g in enumerate(orders[cidx]):
            out[g * P : (g + 1) * P] = yc[ti * P : (ti + 1) * P]
    return out.reshape(B, H, S, S)
